# revision 14
# baseline (speedup 1.0000x reference)
"""Bass kernel for nn_CausalAttention: B=2, L=2048, C=1024, H=16, hd=64 on 8 cores.

Sharding: 2 heads per core (tensor parallel). Each core computes qkv for its
heads, RoPE, causal attention, and a partial projection (its 128 channels x
full Wproj rows slice) -> [4096, 1024] partial (bf16). Host sums partials.

v2: bf16 datapath. Per-core layouts (heads h0, h1):
  qT/kT [128, 4096] bf16: rows = [h0-even d, h0-odd d, h1-even, h1-odd]
     (host permutation of Wq columns), tokens = b*2048 + l.
  vT [128, 4096] bf16: rows = [h0 d(64), h1 d(64)].
  RoPE on DVE from bf16 SBUF raws (2x mode).
  v-nat via DMA transpose (xbar): vT 128x128 blocks -> [128 tok, 16 kt, 132]
     with ones at cols 64/130 (pre-memset) for fused sumexp.
  scores: pair psum [128, 2, 512] f32 (2 k-tiles per exp); causal mask added
     pre-exp by matmul (mw1 @ mw2); one ACT exp per pair -> att bf16.
  AV: psum [65, 512] += matmul(vnat[:, kt, h*66:+65], att[:, j, off:])
  normalize: recip(row 64) -> bcast via K=1 matmul -> DVE mult -> att_sb bf16
  proj: psum [128, 512] = matmul(att_sb[:, mt*128:+128], wp[:, nn*512:+512]);
     copies ACT/DVE alternating into [128, 1024] stage, DMA out per mt.
"""
import math
import numpy as np
from contextlib import ExitStack

import ml_dtypes
import concourse.bass as bass
import concourse.mybir as mybir
import concourse.tile as tile
from concourse.vector_clock import ScopedClock

F32 = mybir.dt.float32
F32R = mybir.dt.float32r
BF16 = mybir.dt.bfloat16
AX = mybir.AluOpType
EXP = mybir.ActivationFunctionType.Exp

B, L, C = 2, 2048, 1024
H, HD = 16, 64
T = B * L          # 4096 tokens
NC_CORES = 8
HPC = H // NC_CORES  # heads per core = 2
QB = 512             # q block
KT = 128             # k tile
NCPB = L // QB       # 4 chunks (q blocks) per batch

NPBF16 = ml_dtypes.bfloat16


# ---------------------------------------------------------------- tile patch
def _patched_drain_and_barrier(self, tick_clock, wait_clock):
    nc = self.nc
    drain_inst = nc.sync.drain()
    wait_clock.add_sem_waits(
        drain_inst.ins, ScopedClock({None: tick_clock.global_clock})
    )
    si = drain_inst.ins.sync_info
    if si is not None and si.on_wait and len(si.on_wait) > 1:
        waits = list(si.on_wait)
        drain_inst.ins.sync_info = mybir.SyncInfo(
            on_wait=waits[:1], on_update=list(si.on_update or [])
        )
        for w in waits[1:]:
            nop = nc.sync.nop(nofuse=True)
            nop.ins.sync_info = mybir.SyncInfo(on_wait=[w], on_update=[])
    nc.all_engine_barrier()
    assert self.sems is not None
    popped = nc._tile_sem_poison_stack.pop()
    assert popped is self._sem_poison
    nc.clear_and_free_semaphores(list(self.sems.allocated().values()))
    nc.all_engine_barrier()


def apply_tile_patch():
    tile.TileContext._drain_and_barrier = _patched_drain_and_barrier


def split_excess_waits(nc, cap=1):
    """Walrus build rejects instructions carrying more than a couple of sync
    waits; move excess waits onto same-engine NoOp carriers inserted right
    before the instruction."""
    for f in nc.m.functions:
        for bb in f.blocks:
            new = []
            for inst in bb.instructions:
                si = inst.sync_info
                waits = list(si.on_wait) if si is not None and si.on_wait else []
                if len(waits) > cap:
                    inst.sync_info = mybir.SyncInfo(
                        on_wait=waits[:cap], on_update=list(si.on_update or []))
                    for w in waits[cap:]:
                        nop = nc.engines[inst.engine].nop(nofuse=True)
                        cur = nc.cur_bb.bb.instructions
                        assert cur and cur[-1].name == nop.ins.name
                        cur.pop()
                        nop.ins.sync_info = mybir.SyncInfo(on_wait=[w], on_update=[])
                        new.append(nop.ins)
                new.append(inst)
            bb.instructions = new


# ---------------------------------------------------------------- host prep
def host_prep():
    """Core-independent prep: rope tables, mask factors."""
    pos = np.arange(L, dtype=np.float64)[:, None]
    dim = np.arange(0, HD, 2, dtype=np.float64)
    freq = pos / (10000.0 ** (dim / HD))      # [L, 32]
    A = np.sin(freq).astype(np.float32)       # 'cos' in ref naming
    Bc = np.cos(freq).astype(np.float32)      # 'sin' in ref naming
    AT = np.ascontiguousarray(A.T)            # [32, L]
    BT = np.ascontiguousarray(Bc.T)
    # TA [128, 4096] = [A;A;A;A] blocks, tokens tiled over batches
    TA = np.tile(AT, (4, B))
    TB = np.tile(np.concatenate([BT, -BT], axis=0), (2, B))  # [+B,-B,+B,-B]
    # mask-add matmul factors: scores += W1^T @ W2kt = -BIG * 1[kp > qf - kt*128]
    BIG = 30.0
    W1 = np.zeros((KT, KT), dtype=np.float32)
    jj = np.arange(KT)[:, None]; kp = np.arange(KT)[None, :]
    W1[:127, :] = -BIG * (kp > jj[:127]).astype(np.float32)
    W1[127, :] = -BIG
    W2 = np.zeros((4, KT, QB), dtype=np.float32)
    qf = np.arange(QB)[None, :]
    for kt in range(4):
        r = qf - kt * KT                       # [1, 512]
        for j in range(127):
            W2[kt, j] = (r[0] == j).astype(np.float32)
        W2[kt, 127] = (r[0] < 0).astype(np.float32)
    return TA, TB, W1, W2


def shard_inputs(x, Wqkv, Wproj):
    """Returns per-core input dicts (bf16 host-side conversion)."""
    x2 = np.ascontiguousarray(x.reshape(T, C))
    xT = np.ascontiguousarray(x2.T).astype(NPBF16)       # [C, T] bf16
    Wq = Wqkv[:, 0 * C:1 * C]
    Wk = Wqkv[:, 1 * C:2 * C]
    Wv = Wqkv[:, 2 * C:3 * C]
    TA, TB, W1, W2 = host_prep()
    scale = 1.0 / math.sqrt(HD)
    perm = np.concatenate([np.arange(0, HD, 2), np.arange(1, HD, 2)])  # even,odd
    in_maps = []
    for c in range(NC_CORES):
        heads = [HPC * c + i for i in range(HPC)]
        qcols = np.concatenate([h * HD + perm for h in heads])
        vcols = np.concatenate([np.arange(h * HD, (h + 1) * HD) for h in heads])
        Wq_c = Wq[:, qcols] * scale           # fold score scale into Wq
        Wk_c = Wk[:, qcols]
        Wv_c = Wv[:, vcols]
        Wqkv_c = np.ascontiguousarray(
            np.concatenate([Wq_c, Wk_c, Wv_c], axis=1))   # [1024, 384]
        Wproj_c = np.ascontiguousarray(Wproj[vcols, :])   # [128, 1024]
        in_maps.append({
            "xT": xT,
            "Wqkv_c": Wqkv_c.astype(NPBF16),
            "Wproj_c": Wproj_c.astype(NPBF16),
            "TA": TA.astype(NPBF16), "TB": TB.astype(NPBF16),
            "maskW1": W1.astype(NPBF16),
            "maskW2": np.ascontiguousarray(
                W2.transpose(1, 0, 2).reshape(KT, 4 * QB)).astype(NPBF16),
            "ones_row": np.ones((1, 64), NPBF16),
        })
    return in_maps


# ---------------------------------------------------------------- kernel build
def build_kernel(debug_outputs=False):
    nc = bass.Bass("TRN2", target_bir_lowering=False, debug=False,
                   num_devices=NC_CORES)
    xT = nc.dram_tensor("xT", [C, T], BF16, kind="ExternalInput")
    Wqkv_c = nc.dram_tensor("Wqkv_c", [C, 3 * 128], BF16, kind="ExternalInput")
    Wproj_c = nc.dram_tensor("Wproj_c", [128, C], BF16, kind="ExternalInput")
    TAd = nc.dram_tensor("TA", [128, T], BF16, kind="ExternalInput")
    TBd = nc.dram_tensor("TB", [128, T], BF16, kind="ExternalInput")
    mw1d = nc.dram_tensor("maskW1", [KT, KT], BF16, kind="ExternalInput")
    mw2d = nc.dram_tensor("maskW2", [KT, 4 * QB], BF16, kind="ExternalInput")
    onesrd = nc.dram_tensor("ones_row", [1, 64], BF16, kind="ExternalInput")
    out = nc.dram_tensor("partial", [T, C], BF16, kind="ExternalOutput")
    dbg = {}
    if debug_outputs:
        dbg["qT"] = nc.dram_tensor("dbg_qT", [128, T], BF16, kind="ExternalOutput")
        dbg["kT"] = nc.dram_tensor("dbg_kT", [128, T], BF16, kind="ExternalOutput")
        dbg["vT"] = nc.dram_tensor("dbg_vT", [128, T], BF16, kind="ExternalOutput")

    with tile.TileContext(nc) as tc, ExitStack() as ctx:
        _build_body(nc, tc, ctx, xT, Wqkv_c, Wproj_c, TAd, TBd, mw1d, mw2d,
                    onesrd, out, dbg)
    return nc


def _build_body(nc, tc, ctx, xT, Wqkv_c, Wproj_c, TAd, TBd, mw1d, mw2d,
                onesrd, out, dbg):
    # ---------------- constants (persistent); wq + first x chunks first
    const = ctx.enter_context(tc.tile_pool(name="const", bufs=1))
    wq = const.tile([128, 8, 384], BF16)
    nc.scalar.dma_start(wq[:], Wqkv_c.ap().rearrange("(o p) f -> p o f", p=128))

    # full x resident in SBUF (bf16, 64KB/partition), loaded per 512-chunk
    xsb_pool = ctx.enter_context(tc.tile_pool(name="xsb", bufs=1))
    xsb = xsb_pool.tile([128, 8, T], BF16)   # [p, o, tok]

    def load_x(nci):
        csl = slice(nci * QB, (nci + 1) * QB)
        nc.sync.dma_start(
            xsb[:, :, csl],
            xT.ap().rearrange("(o p) t -> p o t", p=128)[:, :, csl])

    load_x(0)
    load_x(1)
    TA = const.tile([128, T], BF16)
    TB = const.tile([128, T], BF16)
    nc.scalar.dma_start(TA[:], TAd.ap())
    nc.scalar.dma_start(TB[:], TBd.ap())
    load_x(2)
    mw1 = const.tile([KT, KT], BF16)
    nc.scalar.dma_start(mw1[:], mw1d.ap())
    mw2 = const.tile([KT, 4, QB], BF16)
    nc.scalar.dma_start(mw2[:], mw2d.ap().rearrange("k (m q) -> k m q", m=4))
    ones_row = const.tile([1, 64], BF16)
    nc.scalar.dma_start(ones_row[:], onesrd.ap())
    load_x(3)
    wp = const.tile([128, 1024], BF16)
    nc.scalar.dma_start(wp[:], Wproj_c.ap())
    for nci in range(4, B * NCPB):
        load_x(nci)

    qkv_sb = ctx.enter_context(tc.tile_pool(name="qkv_sb", bufs=1))
    qT = qkv_sb.tile([128, T], BF16)    # rows: h0e,h0o,h1e,h1o (roped)
    kT = qkv_sb.tile([128, T], BF16)
    vT = qkv_sb.tile([128, T], BF16)    # rows: h0 d, h1 d

    vn_pool = ctx.enter_context(tc.tile_pool(name="vnat", bufs=1))
    raw_pool = ctx.enter_context(tc.tile_pool(name="raw", bufs=3))
    att_pool = ctx.enter_context(tc.tile_pool(name="att", bufs=3))
    asb_pool = ctx.enter_context(tc.tile_pool(name="asb", bufs=2))
    rec_pool = ctx.enter_context(tc.tile_pool(name="rec", bufs=2))
    pjs_pool = ctx.enter_context(tc.tile_pool(name="pjs", bufs=2))

    # PSUM: pair tags (2 banks each, bufs=2 -> 4 banks) + av (2) + misc (2)
    psA = ctx.enter_context(tc.tile_pool(name="psA", bufs=2, space="PSUM"))
    psB = ctx.enter_context(tc.tile_pool(name="psB", bufs=2, space="PSUM"))

    vnat = [None, None]
    for b in range(B):
        vn = vn_pool.tile([128, 16, 132], BF16, tag=f"vn{b}")
        vnat[b] = vn
        # ones columns (64, 130) via full memset; transposes overwrite data cols
        nc.gpsimd.memset(vn[:], 1.0)

    def stage_qkv(b, ncil):
        """qkv matmuls + q/k psum->sbuf copies for chunk (b, ncil)."""
        nci = b * NCPB + ncil
        csl = slice(nci * QB, (nci + 1) * QB)
        raws = []
        qk_ps = psA.tile([128, 2, QB], F32, tag="sc", bufs=2)
        for m in range(2):
            for k in range(8):
                nc.tensor.matmul(qk_ps[:, m, :], wq[:, k, m * 128:(m + 1) * 128],
                                 xsb[:, k, csl], start=(k == 0), stop=(k == 7)).annotate('qkmm')
            raw = raw_pool.tile([128, QB], BF16, tag="raw")
            nc.vector.tensor_copy(raw[:], qk_ps[:, m, :])
            raws.append(raw)
        vp = psA.tile([128, 2, QB], F32, tag="sc", bufs=2)
        v_ps = vp[:, 0, :]
        for k in range(8):
            nc.tensor.matmul(v_ps, wq[:, k, 256:384],
                             xsb[:, k, csl], start=(k == 0), stop=(k == 7)).annotate('vmm')
        return raws, v_ps

    def stage_vcopy(b, ncil, v_ps):
        """v psum->sbuf copy + v-nat DMA transposes."""
        vn = vnat[b]
        nci = b * NCPB + ncil
        csl = slice(nci * QB, (nci + 1) * QB)
        nc.scalar.copy(vT[:, csl], v_ps[:])
        for kt in range(ncil * 4, ncil * 4 + 4):
            src = vT[:, b * L + kt * KT: b * L + (kt + 1) * KT]
            dst = vn[:, kt, 0:132].rearrange("p (h w) -> p h w", h=2)[:, :, 0:64]
            nc.sync.dma_start_transpose(dst, src)

    def stage_rope(b, ncil, raws):
        """rope for chunk (b, ncil); q first so scores can start early."""
        nci = b * NCPB + ncil
        csl = slice(nci * QB, (nci + 1) * QB)
        for m, t_ in ((0, qT), (1, kT)):
            raw = raws[m]
            for g in range(4):
                src = (g ^ 1) * 32
                dst = g * 32
                nc.vector.tensor_mul(t_[dst:dst + 32, csl],
                                     raw[src:src + 32, :],
                                     TB[src:src + 32, csl])
            nc.vector.tensor_mul(raw[:], raw[:], TA[:, csl])
            nc.vector.tensor_add(t_[:, csl], t_[:, csl], raw[:])

    def stage_proj(b, qb, att_sb):
        """proj + out DMA for q block (b, qb); copies on ACT."""
        for mt in range(QB // 128):
            row0 = qb * QB + mt * 128
            pj = pjs_pool.tile([128, 1024], BF16, tag="pjs")
            for nn_ in range(2):
                ps = psB.tile([128, QB], F32, tag="ps", bufs=4)
                nc.tensor.matmul(ps[:], att_sb[:, mt * 128:(mt + 1) * 128],
                                 wp[:, nn_ * 512:(nn_ + 1) * 512],
                                 start=True, stop=True).annotate('pjmm')
                if nn_ == 0:
                    nc.scalar.copy(pj[:, 0:512], ps[:])
                else:
                    nc.vector.tensor_copy(pj[:, 512:1024], ps[:])
            nc.sync.dma_start(out.ap()[b * L + row0: b * L + row0 + 128, :], pj[:])

    def stage_b(b, qb):
        """scores + paired exp + AV (skewed) + per-head normalize.

        Score tiles rotate through the persistent 4-bank scring (slot =
        kt % 4); exps cover bank PAIRS (one ACT op per 2 k-tiles, cutting
        the per-op overhead) except diag pairs which exp per-tile at their
        offsets. AV runs 2 tiles behind scores. Returns att_sb."""
        vn = vnat[b]
        nkt = (qb + 1) * (QB // KT)     # causal k tiles
        SKEW = 2
        att_sb = asb_pool.tile([128, QB], BF16, tag="att_sb")
        for h in range(HPC):
            base = h * 64
            aps = psB.tile([128, QB], F32, tag="ps", bufs=4)
            ats = [None] * nkt
            offs = [None] * nkt
            sc = None
            for idx in range(nkt + SKEW):
                if idx < nkt:
                    kt = idx
                    j = kt % 2
                    if j == 0:
                        sc = psA.tile([128, 2, QB], F32, tag="sc", bufs=2)
                    k_sl = slice(b * L + kt * KT, b * L + (kt + 1) * KT)
                    diag = kt - qb * (QB // KT)
                    off = max(0, diag) * KT
                    offs[kt] = off
                    q_sl2 = slice(b * L + qb * QB + off, b * L + (qb + 1) * QB)
                    nc.tensor.matmul(sc[:, j, off:], kT[base:base + 64, k_sl],
                                     qT[base:base + 64, q_sl2],
                                     start=True, stop=(diag < 0)).annotate('scmm')
                    if diag >= 0:
                        nc.tensor.matmul(sc[:, j, off:], mw1[:], mw2[:, diag, off:],
                                         start=False, stop=True).annotate('maskmm')
                    if j == 1:
                        at = att_pool.tile([128, 2, QB], BF16, tag="att")
                        o0, o1 = offs[kt - 1], offs[kt]
                        if o0 == 0 and o1 == 0:
                            nc.scalar.activation(at[:], sc[:], EXP)
                        else:
                            nc.scalar.activation(at[:, 0, o0:], sc[:, 0, o0:], EXP)
                            nc.scalar.activation(at[:, 1, o1:], sc[:, 1, o1:], EXP)
                        ats[kt - 1] = at[:, 0, :]
                        ats[kt] = at[:, 1, :]
                if idx >= SKEW:
                    kt = idx - SKEW
                    off = offs[kt]
                    nc.tensor.matmul(aps[0:65, off:],
                                     vn[:, kt, h * 66:h * 66 + 65],
                                     ats[kt][:, off:],
                                     start=(kt == 0), stop=(kt == nkt - 1)).annotate('avmm')
            # normalize head h; overlaps the other head's scores on PE/ACT
            rec = rec_pool.tile([1, QB], BF16, tag="rec")
            with nc.allow_low_precision(reason="softmax recip to bf16"):
                nc.vector.reciprocal(rec[:], aps[64:65, :])
            bcp = psB.tile([128, QB], F32, tag="ps", bufs=4)
            nc.tensor.matmul(bcp[0:64, :], ones_row[:], rec[:],
                             start=True, stop=True).annotate('bcmm')
            nc.vector.tensor_mul(att_sb[base:base + 64, :],
                                 aps[0:64, :], bcp[0:64, :])
        return att_sb

    # software pipeline per iteration i:
    #   qkv(i) | proj(i-1) | vcopy(i) | rope(i) | scores/exp/AV+norm(i)
    # PE: qkv mms -> proj mms (att_sb(i-1) ready) -> scores;
    # ACT: pj copies -> v copy -> exps; DVE: raw copies -> rope -> norm.
    pending = None     # (b, qb, att_sb) awaiting proj
    for b in range(B):
        for ncil in range(NCPB):
            raws, v_ps = stage_qkv(b, ncil)
            if pending is not None:
                stage_proj(*pending)
            stage_vcopy(b, ncil, v_ps)
            stage_rope(b, ncil, raws)
            att_sb = stage_b(b, ncil)
            pending = (b, ncil, att_sb)
    stage_proj(*pending)
    if dbg:
        nc.sync.dma_start(dbg["qT"].ap(), qT[:])
        nc.sync.dma_start(dbg["kT"].ap(), kT[:])
        nc.sync.dma_start(dbg["vT"].ap(), vT[:])


# ---------------------------------------------------------------- entry point
_NC_CACHE = None
_APPLIED = False


def _ensure_patch():
    global _APPLIED
    if not _APPLIED:
        apply_tile_patch()
        _APPLIED = True


def kernel(x, Wqkv, Wproj):
    """Full-input causal attention on 8 NeuronCores (2 heads per core).

    Each core computes qkv+RoPE+causal attention for its 2 heads and a
    partial projection over its 128 channels; the host sums the 8 partial
    projections (the tensor-parallel all-reduce) and reshapes.
    """
    from concourse.bass_utils import run_bass_kernel_spmd

    global _NC_CACHE
    _ensure_patch()
    x = np.ascontiguousarray(np.asarray(x, dtype=np.float32))
    Wqkv = np.ascontiguousarray(np.asarray(Wqkv, dtype=np.float32))
    Wproj = np.ascontiguousarray(np.asarray(Wproj, dtype=np.float32))
    in_maps = shard_inputs(x, Wqkv, Wproj)
    if _NC_CACHE is None:
        nc = build_kernel(debug_outputs=False)
        split_excess_waits(nc)
        _NC_CACHE = nc
    nc = _NC_CACHE
    res = run_bass_kernel_spmd(nc, in_maps, core_ids=list(range(NC_CORES)))
    acc = np.zeros((T, C), np.float64)
    for r in res.results:
        acc += np.asarray(r["partial"]).astype(np.float64)
    return acc.reshape(B, L, C).astype(np.float32)


# revision 15
# speedup vs baseline: 1.0545x; 1.0545x over previous
"""Bass kernel for nn_CausalAttention: B=2, L=2048, C=1024, H=16, hd=64 on 8 cores.

Sharding: 2 heads per core (tensor parallel). Each core computes qkv for its
heads, RoPE, causal attention, and a partial projection (its 128 channels x
full Wproj rows slice) -> [4096, 1024] partial (bf16). Host sums partials.

v2: bf16 datapath. Per-core layouts (heads h0, h1):
  qT/kT [128, 4096] bf16: rows = [h0-even d, h0-odd d, h1-even, h1-odd]
     (host permutation of Wq columns), tokens = b*2048 + l.
  vT [128, 4096] bf16: rows = [h0 d(64), h1 d(64)].
  RoPE on DVE from bf16 SBUF raws (2x mode).
  v-nat via DMA transpose (xbar): vT 128x128 blocks -> [128 tok, 16 kt, 132]
     with ones at cols 64/130 (pre-memset) for fused sumexp.
  scores: pair psum [128, 2, 512] f32 (2 k-tiles per exp); causal mask added
     pre-exp by matmul (mw1 @ mw2); one ACT exp per pair -> att bf16.
  AV: psum [65, 512] += matmul(vnat[:, kt, h*66:+65], att[:, j, off:])
  normalize: recip(row 64) -> bcast via K=1 matmul -> DVE mult -> att_sb bf16
  proj: psum [128, 512] = matmul(att_sb[:, mt*128:+128], wp[:, nn*512:+512]);
     copies ACT/DVE alternating into [128, 1024] stage, DMA out per mt.
"""
import math
import numpy as np
from contextlib import ExitStack

import ml_dtypes
import concourse.bass as bass
import concourse.mybir as mybir
import concourse.tile as tile
from concourse.vector_clock import ScopedClock

F32 = mybir.dt.float32
F32R = mybir.dt.float32r
BF16 = mybir.dt.bfloat16
AX = mybir.AluOpType
EXP = mybir.ActivationFunctionType.Exp

B, L, C = 2, 2048, 1024
H, HD = 16, 64
T = B * L          # 4096 tokens
NC_CORES = 8
HPC = H // NC_CORES  # heads per core = 2
QB = 512             # q block
KT = 128             # k tile
NCPB = L // QB       # 4 chunks (q blocks) per batch

NPBF16 = ml_dtypes.bfloat16


# ---------------------------------------------------------------- tile patch
def _patched_drain_and_barrier(self, tick_clock, wait_clock):
    nc = self.nc
    drain_inst = nc.sync.drain()
    wait_clock.add_sem_waits(
        drain_inst.ins, ScopedClock({None: tick_clock.global_clock})
    )
    si = drain_inst.ins.sync_info
    if si is not None and si.on_wait and len(si.on_wait) > 1:
        waits = list(si.on_wait)
        drain_inst.ins.sync_info = mybir.SyncInfo(
            on_wait=waits[:1], on_update=list(si.on_update or [])
        )
        for w in waits[1:]:
            nop = nc.sync.nop(nofuse=True)
            nop.ins.sync_info = mybir.SyncInfo(on_wait=[w], on_update=[])
    nc.all_engine_barrier()
    assert self.sems is not None
    popped = nc._tile_sem_poison_stack.pop()
    assert popped is self._sem_poison
    nc.clear_and_free_semaphores(list(self.sems.allocated().values()))
    nc.all_engine_barrier()


def apply_tile_patch():
    tile.TileContext._drain_and_barrier = _patched_drain_and_barrier


def split_excess_waits(nc, cap=1):
    """Walrus build rejects instructions carrying more than a couple of sync
    waits; move excess waits onto same-engine NoOp carriers inserted right
    before the instruction."""
    for f in nc.m.functions:
        for bb in f.blocks:
            new = []
            for inst in bb.instructions:
                si = inst.sync_info
                waits = list(si.on_wait) if si is not None and si.on_wait else []
                if len(waits) > cap:
                    inst.sync_info = mybir.SyncInfo(
                        on_wait=waits[:cap], on_update=list(si.on_update or []))
                    for w in waits[cap:]:
                        nop = nc.engines[inst.engine].nop(nofuse=True)
                        cur = nc.cur_bb.bb.instructions
                        assert cur and cur[-1].name == nop.ins.name
                        cur.pop()
                        nop.ins.sync_info = mybir.SyncInfo(on_wait=[w], on_update=[])
                        new.append(nop.ins)
                new.append(inst)
            bb.instructions = new


# ---------------------------------------------------------------- host prep
def host_prep():
    """Core-independent prep: rope tables, mask factors."""
    pos = np.arange(L, dtype=np.float64)[:, None]
    dim = np.arange(0, HD, 2, dtype=np.float64)
    freq = pos / (10000.0 ** (dim / HD))      # [L, 32]
    A = np.sin(freq).astype(np.float32)       # 'cos' in ref naming
    Bc = np.cos(freq).astype(np.float32)      # 'sin' in ref naming
    AT = np.ascontiguousarray(A.T)            # [32, L]
    BT = np.ascontiguousarray(Bc.T)
    # TA [128, 4096] = [A;A;A;A] blocks, tokens tiled over batches
    TA = np.tile(AT, (4, B))
    TB = np.tile(np.concatenate([BT, -BT], axis=0), (2, B))  # [+B,-B,+B,-B]
    # mask-add matmul factors: scores += W1^T @ W2kt = -BIG * 1[kp > qf - kt*128]
    BIG = 30.0
    W1 = np.zeros((KT, KT), dtype=np.float32)
    jj = np.arange(KT)[:, None]; kp = np.arange(KT)[None, :]
    W1[:127, :] = -BIG * (kp > jj[:127]).astype(np.float32)
    W1[127, :] = -BIG
    W2 = np.zeros((4, KT, QB), dtype=np.float32)
    qf = np.arange(QB)[None, :]
    for kt in range(4):
        r = qf - kt * KT                       # [1, 512]
        for j in range(127):
            W2[kt, j] = (r[0] == j).astype(np.float32)
        W2[kt, 127] = (r[0] < 0).astype(np.float32)
    return TA, TB, W1, W2


def shard_inputs(x, Wqkv, Wproj):
    """Returns per-core input dicts (bf16 host-side conversion)."""
    x2 = np.ascontiguousarray(x.reshape(T, C))
    xT = np.ascontiguousarray(x2.T).astype(NPBF16)       # [C, T] bf16
    Wq = Wqkv[:, 0 * C:1 * C]
    Wk = Wqkv[:, 1 * C:2 * C]
    Wv = Wqkv[:, 2 * C:3 * C]
    TA, TB, W1, W2 = host_prep()
    scale = 1.0 / math.sqrt(HD)
    perm = np.concatenate([np.arange(0, HD, 2), np.arange(1, HD, 2)])  # even,odd
    in_maps = []
    for c in range(NC_CORES):
        heads = [HPC * c + i for i in range(HPC)]
        qcols = np.concatenate([h * HD + perm for h in heads])
        vcols = np.concatenate([np.arange(h * HD, (h + 1) * HD) for h in heads])
        Wq_c = Wq[:, qcols] * scale           # fold score scale into Wq
        Wk_c = Wk[:, qcols]
        Wv_c = Wv[:, vcols]
        Wqkv_c = np.ascontiguousarray(
            np.concatenate([Wq_c, Wk_c, Wv_c], axis=1))   # [1024, 384]
        Wproj_c = np.ascontiguousarray(Wproj[vcols, :])   # [128, 1024]
        in_maps.append({
            "xT": xT,
            "Wqkv_c": Wqkv_c.astype(NPBF16),
            "Wproj_c": Wproj_c.astype(NPBF16),
            "TA": TA.astype(NPBF16), "TB": TB.astype(NPBF16),
            "maskW1": W1.astype(NPBF16),
            "maskW2": np.ascontiguousarray(
                W2.transpose(1, 0, 2).reshape(KT, 4 * QB)).astype(NPBF16),
            "ones_row": np.ones((1, 64), NPBF16),
        })
    return in_maps


# ---------------------------------------------------------------- kernel build
def build_kernel(debug_outputs=False):
    nc = bass.Bass("TRN2", target_bir_lowering=False, debug=False,
                   num_devices=NC_CORES)
    xT = nc.dram_tensor("xT", [C, T], BF16, kind="ExternalInput")
    Wqkv_c = nc.dram_tensor("Wqkv_c", [C, 3 * 128], BF16, kind="ExternalInput")
    Wproj_c = nc.dram_tensor("Wproj_c", [128, C], BF16, kind="ExternalInput")
    TAd = nc.dram_tensor("TA", [128, T], BF16, kind="ExternalInput")
    TBd = nc.dram_tensor("TB", [128, T], BF16, kind="ExternalInput")
    mw1d = nc.dram_tensor("maskW1", [KT, KT], BF16, kind="ExternalInput")
    mw2d = nc.dram_tensor("maskW2", [KT, 4 * QB], BF16, kind="ExternalInput")
    onesrd = nc.dram_tensor("ones_row", [1, 64], BF16, kind="ExternalInput")
    out = nc.dram_tensor("partial", [T, C], BF16, kind="ExternalOutput")
    dbg = {}
    if debug_outputs:
        dbg["qT"] = nc.dram_tensor("dbg_qT", [128, T], BF16, kind="ExternalOutput")
        dbg["kT"] = nc.dram_tensor("dbg_kT", [128, T], BF16, kind="ExternalOutput")
        dbg["vT"] = nc.dram_tensor("dbg_vT", [128, T], BF16, kind="ExternalOutput")

    with tile.TileContext(nc) as tc, ExitStack() as ctx:
        _build_body(nc, tc, ctx, xT, Wqkv_c, Wproj_c, TAd, TBd, mw1d, mw2d,
                    onesrd, out, dbg)
    return nc


def _build_body(nc, tc, ctx, xT, Wqkv_c, Wproj_c, TAd, TBd, mw1d, mw2d,
                onesrd, out, dbg):
    # ---------------- constants (persistent); wq + first x chunks first
    const = ctx.enter_context(tc.tile_pool(name="const", bufs=1))
    wq = const.tile([128, 8, 384], BF16)
    nc.scalar.dma_start(wq[:], Wqkv_c.ap().rearrange("(o p) f -> p o f", p=128))

    # full x resident in SBUF (bf16, 64KB/partition), loaded per 512-chunk
    xsb_pool = ctx.enter_context(tc.tile_pool(name="xsb", bufs=1))
    xsb = xsb_pool.tile([128, 8, T], BF16)   # [p, o, tok]

    def load_x(nci):
        csl = slice(nci * QB, (nci + 1) * QB)
        nc.sync.dma_start(
            xsb[:, :, csl],
            xT.ap().rearrange("(o p) t -> p o t", p=128)[:, :, csl])

    load_x(0)
    load_x(1)
    TA = const.tile([128, T], BF16)
    TB = const.tile([128, T], BF16)
    nc.scalar.dma_start(TA[:], TAd.ap())
    nc.scalar.dma_start(TB[:], TBd.ap())
    load_x(2)
    mw1 = const.tile([KT, KT], BF16)
    nc.scalar.dma_start(mw1[:], mw1d.ap())
    mw2 = const.tile([KT, 4, QB], BF16)
    nc.scalar.dma_start(mw2[:], mw2d.ap().rearrange("k (m q) -> k m q", m=4))
    ones_row = const.tile([1, 64], BF16)
    nc.scalar.dma_start(ones_row[:], onesrd.ap())
    load_x(3)
    wp = const.tile([128, 1024], BF16)
    nc.scalar.dma_start(wp[:], Wproj_c.ap())
    for nci in range(4, B * NCPB):
        load_x(nci)

    qkv_sb = ctx.enter_context(tc.tile_pool(name="qkv_sb", bufs=1))
    qT = qkv_sb.tile([128, T], BF16)    # rows: h0e,h0o,h1e,h1o (roped)
    kT = qkv_sb.tile([128, T], BF16)
    vT = qkv_sb.tile([128, T], BF16)    # rows: h0 d, h1 d

    vn_pool = ctx.enter_context(tc.tile_pool(name="vnat", bufs=1))
    raw_pool = ctx.enter_context(tc.tile_pool(name="raw", bufs=3))
    att_pool = ctx.enter_context(tc.tile_pool(name="att", bufs=3))
    asb_pool = ctx.enter_context(tc.tile_pool(name="asb", bufs=2))
    rec_pool = ctx.enter_context(tc.tile_pool(name="rec", bufs=2))
    pjs_pool = ctx.enter_context(tc.tile_pool(name="pjs", bufs=2))

    # PSUM: pair tags (2 banks each, bufs=2 -> 4 banks) + av (2) + misc (2)
    psA = ctx.enter_context(tc.tile_pool(name="psA", bufs=2, space="PSUM"))
    psB = ctx.enter_context(tc.tile_pool(name="psB", bufs=2, space="PSUM"))

    vnat = [None, None]
    for b in range(B):
        vn = vn_pool.tile([128, 16, 132], BF16, tag=f"vn{b}")
        vnat[b] = vn
        # ones columns (64, 130) via full memset; transposes overwrite data cols
        nc.gpsimd.memset(vn[:], 1.0)

    def stage_qkv(b, ncil):
        """qkv matmuls + q/k psum->sbuf copies for chunk (b, ncil)."""
        nci = b * NCPB + ncil
        csl = slice(nci * QB, (nci + 1) * QB)
        raws = []
        qk_ps = psA.tile([128, 2, QB], F32, tag="sc", bufs=2)
        for m in range(2):
            for k in range(8):
                nc.tensor.matmul(qk_ps[:, m, :], wq[:, k, m * 128:(m + 1) * 128],
                                 xsb[:, k, csl], start=(k == 0), stop=(k == 7)).annotate('qkmm')
            raw = raw_pool.tile([128, QB], BF16, tag="raw")
            nc.vector.tensor_copy(raw[:], qk_ps[:, m, :])
            raws.append(raw)
        vp = psA.tile([128, 2, QB], F32, tag="sc", bufs=2)
        v_ps = vp[:, 0, :]
        for k in range(8):
            nc.tensor.matmul(v_ps, wq[:, k, 256:384],
                             xsb[:, k, csl], start=(k == 0), stop=(k == 7)).annotate('vmm')
        return raws, v_ps

    def stage_vcopy(b, ncil, v_ps):
        """v psum->sbuf copy + v-nat DMA transposes."""
        vn = vnat[b]
        nci = b * NCPB + ncil
        csl = slice(nci * QB, (nci + 1) * QB)
        nc.scalar.copy(vT[:, csl], v_ps[:])
        for kt in range(ncil * 4, ncil * 4 + 4):
            src = vT[:, b * L + kt * KT: b * L + (kt + 1) * KT]
            dst = vn[:, kt, 0:132].rearrange("p (h w) -> p h w", h=2)[:, :, 0:64]
            nc.sync.dma_start_transpose(dst, src)

    def stage_rope(b, ncil, raws):
        """rope for chunk (b, ncil); q first so scores can start early."""
        nci = b * NCPB + ncil
        csl = slice(nci * QB, (nci + 1) * QB)
        for m, t_ in ((0, qT), (1, kT)):
            raw = raws[m]
            for g in range(4):
                src = (g ^ 1) * 32
                dst = g * 32
                nc.vector.tensor_mul(t_[dst:dst + 32, csl],
                                     raw[src:src + 32, :],
                                     TB[src:src + 32, csl])
            nc.vector.tensor_mul(raw[:], raw[:], TA[:, csl])
            nc.vector.tensor_add(t_[:, csl], t_[:, csl], raw[:])

    def stage_proj(b, qb, att_sb):
        """proj + out DMA for q block (b, qb); copies on ACT."""
        for mt in range(QB // 128):
            row0 = qb * QB + mt * 128
            pj = pjs_pool.tile([128, 1024], BF16, tag="pjs")
            for nn_ in range(2):
                ps = psB.tile([128, QB], F32, tag="ps", bufs=4)
                nc.tensor.matmul(ps[:], att_sb[:, mt * 128:(mt + 1) * 128],
                                 wp[:, nn_ * 512:(nn_ + 1) * 512],
                                 start=True, stop=True).annotate('pjmm')
                if nn_ == 0:
                    nc.scalar.copy(pj[:, 0:512], ps[:])
                else:
                    nc.vector.tensor_copy(pj[:, 512:1024], ps[:])
            nc.sync.dma_start(out.ap()[b * L + row0: b * L + row0 + 128, :], pj[:])

    def stage_b(b, qb):
        """scores + paired exp + AV (skewed) + per-head normalize.

        Score tiles rotate through the persistent 4-bank scring (slot =
        kt % 4); exps cover bank PAIRS (one ACT op per 2 k-tiles, cutting
        the per-op overhead) except diag pairs which exp per-tile at their
        offsets. AV runs 2 tiles behind scores. Returns att_sb."""
        vn = vnat[b]
        nkt = (qb + 1) * (QB // KT)     # causal k tiles
        SKEW = 4
        att_sb = asb_pool.tile([128, QB], BF16, tag="att_sb")
        for h in range(HPC):
            base = h * 64
            aps = psB.tile([128, QB], F32, tag="ps", bufs=4)
            ats = [None] * nkt
            offs = [None] * nkt
            sc = None
            for idx in range(nkt + SKEW):
                if idx < nkt:
                    kt = idx
                    j = kt % 2
                    if j == 0:
                        sc = psA.tile([128, 2, QB], F32, tag="sc", bufs=2)
                    k_sl = slice(b * L + kt * KT, b * L + (kt + 1) * KT)
                    diag = kt - qb * (QB // KT)
                    off = max(0, diag) * KT
                    offs[kt] = off
                    q_sl2 = slice(b * L + qb * QB + off, b * L + (qb + 1) * QB)
                    nc.tensor.matmul(sc[:, j, off:], kT[base:base + 64, k_sl],
                                     qT[base:base + 64, q_sl2],
                                     start=True, stop=(diag < 0)).annotate('scmm')
                    if diag >= 0:
                        nc.tensor.matmul(sc[:, j, off:], mw1[:], mw2[:, diag, off:],
                                         start=False, stop=True).annotate('maskmm')
                    if j == 1:
                        at = att_pool.tile([128, 2, QB], BF16, tag="att")
                        o0, o1 = offs[kt - 1], offs[kt]
                        if o0 == 0 and o1 == 0:
                            nc.scalar.activation(at[:], sc[:], EXP)
                        else:
                            nc.scalar.activation(at[:, 0, o0:], sc[:, 0, o0:], EXP)
                            nc.scalar.activation(at[:, 1, o1:], sc[:, 1, o1:], EXP)
                        ats[kt - 1] = at[:, 0, :]
                        ats[kt] = at[:, 1, :]
                if idx >= SKEW:
                    kt = idx - SKEW
                    off = offs[kt]
                    nc.tensor.matmul(aps[0:65, off:],
                                     vn[:, kt, h * 66:h * 66 + 65],
                                     ats[kt][:, off:],
                                     start=(kt == 0), stop=(kt == nkt - 1)).annotate('avmm')
            # normalize head h; overlaps the other head's scores on PE/ACT
            rec = rec_pool.tile([1, QB], BF16, tag="rec")
            with nc.allow_low_precision(reason="softmax recip to bf16"):
                nc.vector.reciprocal(rec[:], aps[64:65, :])
            bcp = psB.tile([128, QB], F32, tag="ps", bufs=4)
            nc.tensor.matmul(bcp[0:64, :], ones_row[:], rec[:],
                             start=True, stop=True).annotate('bcmm')
            nc.vector.tensor_mul(att_sb[base:base + 64, :],
                                 aps[0:64, :], bcp[0:64, :])
        return att_sb

    # software pipeline per iteration i:
    #   qkv(i) | proj(i-1) | vcopy(i) | rope(i) | scores/exp/AV+norm(i)
    # PE: qkv mms -> proj mms (att_sb(i-1) ready) -> scores;
    # ACT: pj copies -> v copy -> exps; DVE: raw copies -> rope -> norm.
    pending = None     # (b, qb, att_sb) awaiting proj
    for b in range(B):
        for ncil in range(NCPB):
            raws, v_ps = stage_qkv(b, ncil)
            if pending is not None:
                stage_proj(*pending)
            stage_vcopy(b, ncil, v_ps)
            stage_rope(b, ncil, raws)
            att_sb = stage_b(b, ncil)
            pending = (b, ncil, att_sb)
    stage_proj(*pending)
    if dbg:
        nc.sync.dma_start(dbg["qT"].ap(), qT[:])
        nc.sync.dma_start(dbg["kT"].ap(), kT[:])
        nc.sync.dma_start(dbg["vT"].ap(), vT[:])


# ---------------------------------------------------------------- entry point
_NC_CACHE = None
_APPLIED = False


def _ensure_patch():
    global _APPLIED
    if not _APPLIED:
        apply_tile_patch()
        _APPLIED = True


def kernel(x, Wqkv, Wproj):
    """Full-input causal attention on 8 NeuronCores (2 heads per core).

    Each core computes qkv+RoPE+causal attention for its 2 heads and a
    partial projection over its 128 channels; the host sums the 8 partial
    projections (the tensor-parallel all-reduce) and reshapes.
    """
    from concourse.bass_utils import run_bass_kernel_spmd

    global _NC_CACHE
    _ensure_patch()
    x = np.ascontiguousarray(np.asarray(x, dtype=np.float32))
    Wqkv = np.ascontiguousarray(np.asarray(Wqkv, dtype=np.float32))
    Wproj = np.ascontiguousarray(np.asarray(Wproj, dtype=np.float32))
    in_maps = shard_inputs(x, Wqkv, Wproj)
    if _NC_CACHE is None:
        nc = build_kernel(debug_outputs=False)
        split_excess_waits(nc)
        _NC_CACHE = nc
    nc = _NC_CACHE
    res = run_bass_kernel_spmd(nc, in_maps, core_ids=list(range(NC_CORES)))
    acc = np.zeros((T, C), np.float64)
    for r in res.results:
        acc += np.asarray(r["partial"]).astype(np.float64)
    return acc.reshape(B, L, C).astype(np.float32)


# revision 16
# speedup vs baseline: 1.2577x; 1.1928x over previous
"""Bass kernel for nn_CausalAttention: B=2, L=2048, C=1024, H=16, hd=64 on 8 cores.

Sharding: 2 heads per core (tensor parallel). Each core computes qkv for its
heads, RoPE, causal attention, and a partial projection (its 128 channels x
full Wproj rows slice) -> [4096, 1024] partial (bf16). Host sums partials.

v2: bf16 datapath. Per-core layouts (heads h0, h1):
  qT/kT [128, 4096] bf16: rows = [h0-even d, h0-odd d, h1-even, h1-odd]
     (host permutation of Wq columns), tokens = b*2048 + l.
  vT [128, 4096] bf16: rows = [h0 d(64), h1 d(64)].
  RoPE on DVE from bf16 SBUF raws (2x mode).
  v-nat via DMA transpose (xbar): vT 128x128 blocks -> [128 tok, 16 kt, 132]
     with ones at cols 64/130 (pre-memset) for fused sumexp.
  scores: pair psum [128, 2, 512] f32 (2 k-tiles per exp); causal mask added
     pre-exp by matmul (mw1 @ mw2); one ACT exp per pair -> att bf16.
  AV: psum [65, 512] += matmul(vnat[:, kt, h*66:+65], att[:, j, off:])
  normalize: recip(row 64) -> bcast via K=1 matmul -> DVE mult -> att_sb bf16
  proj: psum [128, 512] = matmul(att_sb[:, mt*128:+128], wp[:, nn*512:+512]);
     copies ACT/DVE alternating into [128, 1024] stage, DMA out per mt.
"""
import math
import numpy as np
from contextlib import ExitStack

import ml_dtypes
import concourse.bass as bass
import concourse.mybir as mybir
import concourse.tile as tile
from concourse.vector_clock import ScopedClock

F32 = mybir.dt.float32
F32R = mybir.dt.float32r
BF16 = mybir.dt.bfloat16
AX = mybir.AluOpType
EXP = mybir.ActivationFunctionType.Exp

B, L, C = 2, 2048, 1024
H, HD = 16, 64
T = B * L          # 4096 tokens
NC_CORES = 8
HPC = H // NC_CORES  # heads per core = 2
QB = 512             # q block
KT = 128             # k tile
NCPB = L // QB       # 4 chunks (q blocks) per batch

NPBF16 = ml_dtypes.bfloat16


# ---------------------------------------------------------------- tile patch
def _patched_drain_and_barrier(self, tick_clock, wait_clock):
    nc = self.nc
    drain_inst = nc.sync.drain()
    wait_clock.add_sem_waits(
        drain_inst.ins, ScopedClock({None: tick_clock.global_clock})
    )
    si = drain_inst.ins.sync_info
    if si is not None and si.on_wait and len(si.on_wait) > 1:
        waits = list(si.on_wait)
        drain_inst.ins.sync_info = mybir.SyncInfo(
            on_wait=waits[:1], on_update=list(si.on_update or [])
        )
        for w in waits[1:]:
            nop = nc.sync.nop(nofuse=True)
            nop.ins.sync_info = mybir.SyncInfo(on_wait=[w], on_update=[])
    nc.all_engine_barrier()
    assert self.sems is not None
    popped = nc._tile_sem_poison_stack.pop()
    assert popped is self._sem_poison
    nc.clear_and_free_semaphores(list(self.sems.allocated().values()))
    nc.all_engine_barrier()


def apply_tile_patch():
    tile.TileContext._drain_and_barrier = _patched_drain_and_barrier


def split_excess_waits(nc, cap=1):
    """Walrus build rejects instructions carrying more than a couple of sync
    waits; move excess waits onto same-engine NoOp carriers inserted right
    before the instruction."""
    for f in nc.m.functions:
        for bb in f.blocks:
            new = []
            for inst in bb.instructions:
                si = inst.sync_info
                waits = list(si.on_wait) if si is not None and si.on_wait else []
                if len(waits) > cap:
                    inst.sync_info = mybir.SyncInfo(
                        on_wait=waits[:cap], on_update=list(si.on_update or []))
                    for w in waits[cap:]:
                        nop = nc.engines[inst.engine].nop(nofuse=True)
                        cur = nc.cur_bb.bb.instructions
                        assert cur and cur[-1].name == nop.ins.name
                        cur.pop()
                        nop.ins.sync_info = mybir.SyncInfo(on_wait=[w], on_update=[])
                        new.append(nop.ins)
                new.append(inst)
            bb.instructions = new


# ---------------------------------------------------------------- host prep
def host_prep():
    """Core-independent prep: rope tables, mask factors."""
    pos = np.arange(L, dtype=np.float64)[:, None]
    dim = np.arange(0, HD, 2, dtype=np.float64)
    freq = pos / (10000.0 ** (dim / HD))      # [L, 32]
    A = np.sin(freq).astype(np.float32)       # 'cos' in ref naming
    Bc = np.cos(freq).astype(np.float32)      # 'sin' in ref naming
    AT = np.ascontiguousarray(A.T)            # [32, L]
    BT = np.ascontiguousarray(Bc.T)
    # TA [128, 4096] = [A;A;A;A] blocks, tokens tiled over batches
    TA = np.tile(AT, (4, B))
    TB = np.tile(np.concatenate([BT, -BT], axis=0), (2, B))  # [+B,-B,+B,-B]
    # mask-add matmul factors: scores += W1^T @ W2kt = -BIG * 1[kp > qf - kt*128]
    BIG = 30.0
    W1 = np.zeros((KT, KT), dtype=np.float32)
    jj = np.arange(KT)[:, None]; kp = np.arange(KT)[None, :]
    W1[:127, :] = -BIG * (kp > jj[:127]).astype(np.float32)
    W1[127, :] = -BIG
    W2 = np.zeros((4, KT, QB), dtype=np.float32)
    qf = np.arange(QB)[None, :]
    for kt in range(4):
        r = qf - kt * KT                       # [1, 512]
        for j in range(127):
            W2[kt, j] = (r[0] == j).astype(np.float32)
        W2[kt, 127] = (r[0] < 0).astype(np.float32)
    return TA, TB, W1, W2


def shard_inputs(x, Wqkv, Wproj):
    """Returns per-core input dicts (bf16 host-side conversion)."""
    x2 = np.ascontiguousarray(x.reshape(T, C))
    xT = np.ascontiguousarray(x2.T).astype(NPBF16)       # [C, T] bf16
    Wq = Wqkv[:, 0 * C:1 * C]
    Wk = Wqkv[:, 1 * C:2 * C]
    Wv = Wqkv[:, 2 * C:3 * C]
    TA, TB, W1, W2 = host_prep()
    scale = 1.0 / math.sqrt(HD)
    perm = np.concatenate([np.arange(0, HD, 2), np.arange(1, HD, 2)])  # even,odd
    in_maps = []
    for c in range(NC_CORES):
        heads = [HPC * c + i for i in range(HPC)]
        qcols = np.concatenate([h * HD + perm for h in heads])
        vcols = np.concatenate([np.arange(h * HD, (h + 1) * HD) for h in heads])
        Wq_c = Wq[:, qcols] * scale           # fold score scale into Wq
        Wk_c = Wk[:, qcols]
        Wv_c = Wv[:, vcols]
        Wqkv_c = np.ascontiguousarray(
            np.concatenate([Wq_c, Wk_c, Wv_c], axis=1))   # [1024, 384]
        Wproj_c = np.ascontiguousarray(Wproj[vcols, :])   # [128, 1024]
        in_maps.append({
            "xT": xT,
            "Wqkv_c": Wqkv_c.astype(NPBF16),
            "Wproj_c": Wproj_c.astype(NPBF16),
            "TA": TA.astype(NPBF16), "TB": TB.astype(NPBF16),
            "maskW1": W1.astype(NPBF16),
            "maskW2": np.ascontiguousarray(
                W2.transpose(1, 0, 2).reshape(KT, 4 * QB)).astype(NPBF16),
            "ones_row": np.ones((1, 64), NPBF16),
        })
    return in_maps


# ---------------------------------------------------------------- kernel build
def build_kernel(debug_outputs=False):
    nc = bass.Bass("TRN2", target_bir_lowering=False, debug=False,
                   num_devices=NC_CORES)
    xT = nc.dram_tensor("xT", [C, T], BF16, kind="ExternalInput")
    Wqkv_c = nc.dram_tensor("Wqkv_c", [C, 3 * 128], BF16, kind="ExternalInput")
    Wproj_c = nc.dram_tensor("Wproj_c", [128, C], BF16, kind="ExternalInput")
    TAd = nc.dram_tensor("TA", [128, T], BF16, kind="ExternalInput")
    TBd = nc.dram_tensor("TB", [128, T], BF16, kind="ExternalInput")
    mw1d = nc.dram_tensor("maskW1", [KT, KT], BF16, kind="ExternalInput")
    mw2d = nc.dram_tensor("maskW2", [KT, 4 * QB], BF16, kind="ExternalInput")
    onesrd = nc.dram_tensor("ones_row", [1, 64], BF16, kind="ExternalInput")
    out = nc.dram_tensor("partial", [T, C], BF16, kind="ExternalOutput")
    dbg = {}
    if debug_outputs:
        dbg["qT"] = nc.dram_tensor("dbg_qT", [128, T], BF16, kind="ExternalOutput")
        dbg["kT"] = nc.dram_tensor("dbg_kT", [128, T], BF16, kind="ExternalOutput")
        dbg["vT"] = nc.dram_tensor("dbg_vT", [128, T], BF16, kind="ExternalOutput")

    with tile.TileContext(nc) as tc, ExitStack() as ctx:
        _build_body(nc, tc, ctx, xT, Wqkv_c, Wproj_c, TAd, TBd, mw1d, mw2d,
                    onesrd, out, dbg)
    return nc


def _build_body(nc, tc, ctx, xT, Wqkv_c, Wproj_c, TAd, TBd, mw1d, mw2d,
                onesrd, out, dbg):
    # ---------------- constants (persistent); wq + first x chunks first
    const = ctx.enter_context(tc.tile_pool(name="const", bufs=1))
    wq = const.tile([128, 8, 384], BF16)
    nc.scalar.dma_start(wq[:], Wqkv_c.ap().rearrange("(o p) f -> p o f", p=128))

    # full x resident in SBUF (bf16, 64KB/partition), loaded per 512-chunk
    xsb_pool = ctx.enter_context(tc.tile_pool(name="xsb", bufs=1))
    xsb = xsb_pool.tile([128, 8, T], BF16)   # [p, o, tok]

    def load_x(nci):
        csl = slice(nci * QB, (nci + 1) * QB)
        nc.sync.dma_start(
            xsb[:, :, csl],
            xT.ap().rearrange("(o p) t -> p o t", p=128)[:, :, csl])

    load_x(0)
    load_x(1)
    TA = const.tile([128, T], BF16)
    TB = const.tile([128, T], BF16)
    nc.scalar.dma_start(TA[:], TAd.ap())
    nc.scalar.dma_start(TB[:], TBd.ap())
    load_x(2)
    mw1 = const.tile([KT, KT], BF16)
    nc.scalar.dma_start(mw1[:], mw1d.ap())
    mw2 = const.tile([KT, 4, QB], BF16)
    nc.scalar.dma_start(mw2[:], mw2d.ap().rearrange("k (m q) -> k m q", m=4))
    ones_row = const.tile([1, 64], BF16)
    nc.scalar.dma_start(ones_row[:], onesrd.ap())
    load_x(3)
    wp = const.tile([128, 1024], BF16)
    nc.scalar.dma_start(wp[:], Wproj_c.ap())
    for nci in range(4, B * NCPB):
        load_x(nci)

    qkv_sb = ctx.enter_context(tc.tile_pool(name="qkv_sb", bufs=1))
    qT = qkv_sb.tile([128, T], BF16)    # rows: h0e,h0o,h1e,h1o (roped)
    kT = qkv_sb.tile([128, T], BF16)
    vT = qkv_sb.tile([128, T], BF16)    # rows: h0 d, h1 d

    vn_pool = ctx.enter_context(tc.tile_pool(name="vnat", bufs=1))
    raw_pool = ctx.enter_context(tc.tile_pool(name="raw", bufs=3))
    att_pool = ctx.enter_context(tc.tile_pool(name="att", bufs=3))
    asb_pool = ctx.enter_context(tc.tile_pool(name="asb", bufs=2))
    rec_pool = ctx.enter_context(tc.tile_pool(name="rec", bufs=2))
    pjs_pool = ctx.enter_context(tc.tile_pool(name="pjs", bufs=4))

    # PSUM: pair tags (2 banks each, bufs=2 -> 4 banks) + av (2) + misc (2)
    psA = ctx.enter_context(tc.tile_pool(name="psA", bufs=2, space="PSUM"))
    psB = ctx.enter_context(tc.tile_pool(name="psB", bufs=2, space="PSUM"))

    vnat = [None, None]
    for b in range(B):
        vn = vn_pool.tile([128, 16, 132], BF16, tag=f"vn{b}")
        vnat[b] = vn
        # ones columns (64, 130) via full memset; transposes overwrite data cols
        nc.gpsimd.memset(vn[:], 1.0)

    def stage_qkv(b, ncil):
        """qkv matmuls + q/k psum->sbuf copies for chunk (b, ncil)."""
        nci = b * NCPB + ncil
        csl = slice(nci * QB, (nci + 1) * QB)
        raws = []
        qk_ps = psA.tile([128, 2, QB], F32, tag="sc", bufs=2)
        for m in range(2):
            for k in range(8):
                nc.tensor.matmul(qk_ps[:, m, :], wq[:, k, m * 128:(m + 1) * 128],
                                 xsb[:, k, csl], start=(k == 0), stop=(k == 7)).annotate('qkmm')
            raw = raw_pool.tile([128, QB], BF16, tag="raw")
            nc.vector.tensor_copy(raw[:], qk_ps[:, m, :])
            raws.append(raw)
        vp = psA.tile([128, 2, QB], F32, tag="sc", bufs=2)
        v_ps = vp[:, 0, :]
        for k in range(8):
            nc.tensor.matmul(v_ps, wq[:, k, 256:384],
                             xsb[:, k, csl], start=(k == 0), stop=(k == 7)).annotate('vmm')
        return raws, v_ps

    def stage_vcopy(b, ncil, v_ps):
        """v psum->sbuf copy + v-nat DMA transposes."""
        vn = vnat[b]
        nci = b * NCPB + ncil
        csl = slice(nci * QB, (nci + 1) * QB)
        nc.scalar.copy(vT[:, csl], v_ps[:])
        for kt in range(ncil * 4, ncil * 4 + 4):
            src = vT[:, b * L + kt * KT: b * L + (kt + 1) * KT]
            dst = vn[:, kt, 0:132].rearrange("p (h w) -> p h w", h=2)[:, :, 0:64]
            nc.sync.dma_start_transpose(dst, src)

    def stage_rope(b, ncil, raws):
        """rope for chunk (b, ncil); q first so scores can start early."""
        nci = b * NCPB + ncil
        csl = slice(nci * QB, (nci + 1) * QB)
        for m, t_ in ((0, qT), (1, kT)):
            raw = raws[m]
            for g in range(4):
                src = (g ^ 1) * 32
                dst = g * 32
                nc.vector.tensor_mul(t_[dst:dst + 32, csl],
                                     raw[src:src + 32, :],
                                     TB[src:src + 32, csl])
            nc.vector.tensor_mul(raw[:], raw[:], TA[:, csl])
            nc.vector.tensor_add(t_[:, csl], t_[:, csl], raw[:])

    def stage_proj(b, qb, att_sb):
        """proj + out DMA for q block (b, qb); copies on ACT."""
        for mt in range(QB // 128):
            row0 = qb * QB + mt * 128
            pj = pjs_pool.tile([128, 1024], BF16, tag="pjs")
            for nn_ in range(2):
                ps = psB.tile([128, QB], F32, tag="misc", bufs=2)
                nc.tensor.matmul(ps[:], att_sb[:, mt * 128:(mt + 1) * 128],
                                 wp[:, nn_ * 512:(nn_ + 1) * 512],
                                 start=True, stop=True).annotate('pjmm')
                if nn_ == 0:
                    nc.scalar.copy(pj[:, 0:512], ps[:])
                else:
                    nc.vector.tensor_copy(pj[:, 512:1024], ps[:])
            nc.gpsimd.dma_start(out.ap()[b * L + row0: b * L + row0 + 128, :], pj[:])

    def stage_b(b, qb):
        """scores + paired exp + AV (skewed) + per-head normalize.

        Score tiles rotate through the persistent 4-bank scring (slot =
        kt % 4); exps cover bank PAIRS (one ACT op per 2 k-tiles, cutting
        the per-op overhead) except diag pairs which exp per-tile at their
        offsets. AV runs 2 tiles behind scores. Returns att_sb."""
        vn = vnat[b]
        nkt = (qb + 1) * (QB // KT)     # causal k tiles
        SKEW = 4
        att_sb = asb_pool.tile([128, QB], BF16, tag="att_sb")
        for h in range(HPC):
            base = h * 64
            aps = psB.tile([128, QB], F32, tag="av", bufs=2)
            ats = [None] * nkt
            offs = [None] * nkt
            sc = None
            for idx in range(nkt + SKEW):
                if idx < nkt:
                    kt = idx
                    j = kt % 2
                    if j == 0:
                        sc = psA.tile([128, 2, QB], F32, tag="sc", bufs=2)
                    k_sl = slice(b * L + kt * KT, b * L + (kt + 1) * KT)
                    diag = kt - qb * (QB // KT)
                    off = max(0, diag) * KT
                    offs[kt] = off
                    q_sl2 = slice(b * L + qb * QB + off, b * L + (qb + 1) * QB)
                    nc.tensor.matmul(sc[:, j, off:], kT[base:base + 64, k_sl],
                                     qT[base:base + 64, q_sl2],
                                     start=True, stop=(diag < 0)).annotate('scmm')
                    if diag >= 0:
                        nc.tensor.matmul(sc[:, j, off:], mw1[:], mw2[:, diag, off:],
                                         start=False, stop=True).annotate('maskmm')
                    if j == 1:
                        at = att_pool.tile([128, 2, QB], BF16, tag="att")
                        o0, o1 = offs[kt - 1], offs[kt]
                        if o0 == 0 and o1 == 0:
                            nc.scalar.activation(at[:], sc[:], EXP)
                        else:
                            nc.scalar.activation(at[:, 0, o0:], sc[:, 0, o0:], EXP)
                            nc.scalar.activation(at[:, 1, o1:], sc[:, 1, o1:], EXP)
                        ats[kt - 1] = at[:, 0, :]
                        ats[kt] = at[:, 1, :]
                if idx >= SKEW:
                    kt = idx - SKEW
                    off = offs[kt]
                    nc.tensor.matmul(aps[0:65, off:],
                                     vn[:, kt, h * 66:h * 66 + 65],
                                     ats[kt][:, off:],
                                     start=(kt == 0), stop=(kt == nkt - 1)).annotate('avmm')
            # normalize head h; overlaps the other head's scores on PE/ACT
            rec = rec_pool.tile([1, QB], BF16, tag="rec")
            with nc.allow_low_precision(reason="softmax recip to bf16"):
                nc.vector.reciprocal(rec[:], aps[64:65, :])
            bcp = psB.tile([128, QB], F32, tag="misc", bufs=2)
            nc.tensor.matmul(bcp[0:64, :], ones_row[:], rec[:],
                             start=True, stop=True).annotate('bcmm')
            nc.vector.tensor_mul(att_sb[base:base + 64, :],
                                 aps[0:64, :], bcp[0:64, :])
        return att_sb

    # software pipeline per iteration i:
    #   qkv(i) | proj(i-1) | vcopy(i) | rope(i) | scores/exp/AV+norm(i)
    # PE: qkv mms -> proj mms (att_sb(i-1) ready) -> scores;
    # ACT: pj copies -> v copy -> exps; DVE: raw copies -> rope -> norm.
    pending = None     # (b, qb, att_sb) awaiting proj
    for b in range(B):
        for ncil in range(NCPB):
            raws, v_ps = stage_qkv(b, ncil)
            if pending is not None:
                stage_proj(*pending)
            stage_vcopy(b, ncil, v_ps)
            stage_rope(b, ncil, raws)
            att_sb = stage_b(b, ncil)
            pending = (b, ncil, att_sb)
    stage_proj(*pending)
    if dbg:
        nc.sync.dma_start(dbg["qT"].ap(), qT[:])
        nc.sync.dma_start(dbg["kT"].ap(), kT[:])
        nc.sync.dma_start(dbg["vT"].ap(), vT[:])


# ---------------------------------------------------------------- entry point
_NC_CACHE = None
_APPLIED = False


def _ensure_patch():
    global _APPLIED
    if not _APPLIED:
        apply_tile_patch()
        _APPLIED = True


def kernel(x, Wqkv, Wproj):
    """Full-input causal attention on 8 NeuronCores (2 heads per core).

    Each core computes qkv+RoPE+causal attention for its 2 heads and a
    partial projection over its 128 channels; the host sums the 8 partial
    projections (the tensor-parallel all-reduce) and reshapes.
    """
    from concourse.bass_utils import run_bass_kernel_spmd

    global _NC_CACHE
    _ensure_patch()
    x = np.ascontiguousarray(np.asarray(x, dtype=np.float32))
    Wqkv = np.ascontiguousarray(np.asarray(Wqkv, dtype=np.float32))
    Wproj = np.ascontiguousarray(np.asarray(Wproj, dtype=np.float32))
    in_maps = shard_inputs(x, Wqkv, Wproj)
    if _NC_CACHE is None:
        nc = build_kernel(debug_outputs=False)
        split_excess_waits(nc)
        _NC_CACHE = nc
    nc = _NC_CACHE
    res = run_bass_kernel_spmd(nc, in_maps, core_ids=list(range(NC_CORES)))
    acc = np.zeros((T, C), np.float64)
    for r in res.results:
        acc += np.asarray(r["partial"]).astype(np.float64)
    return acc.reshape(B, L, C).astype(np.float32)
